# revision 1
# baseline (speedup 1.0000x reference)
"""MinHash sketch kernel for Trainium2 (8 NeuronCores, Bass/Tile).

Computes: sketch = segment_min(x @ hash_matrices.T, batch) over 512 segments,
with empty segments set to 0.  x: [N, 256] f32, batch: [N] sorted int64,
hash_matrices: [128, 256] f32 -> out [512, 128] f32.

Strategy (data-parallel over nodes):
  * Host groups each segment's nodes into 128-wide "groups" (the last group of
    a segment is padded with duplicate nodes from the same segment, which is
    min-neutral).  Groups are distributed contiguously over the 8 cores and
    padded with dummy groups so every core runs the identical program.
  * Each core's node shard is laid out TRANSPOSED on host ([256, cols], f32) so
    the contraction dim (features) sits on SBUF partitions - no on-device
    transpose needed.
  * Device: hv[h, n] = (H^T chunk0).T @ xT chunk0 + (H^T chunk1).T @ xT chunk1
    accumulated in PSUM (h = 128 hashes on partitions), then one segmented
    reduce_min per PSUM bank ([128, 4, 128] -> [128, 4]) into an SBUF
    accumulator [128, G]; a single DMA writes it out.
  * Host: scatter-min each (core, group) column back to its segment, zero empty
    segments.
"""

import sys

if "/opt/trn_rl_repo" not in sys.path:
    sys.path.insert(0, "/opt/trn_rl_repo")

import numpy as np

N_CORES = 8
W = 128          # nodes per group (reduce_min granularity)
TB = 2048        # columns per DMA block (1 MiB per 128-partition chunk DMA)
NUM_HASHES = 128
FEATURE_DIM = 256

_compiled_cache = {}


def _build_program(cols):
    """Build + compile the single-core Bass program for a shard of `cols`
    node-columns (cols % TB == 0).  Returns the compiled Bacc module."""
    import concourse.bacc as bacc
    import concourse.mybir as mybir
    import concourse.tile as tile

    nc = bacc.Bacc("TRN2", target_bir_lowering=False, debug=False,
                   num_devices=N_CORES)

    xt = nc.dram_tensor("xt", [FEATURE_DIM, cols], mybir.dt.float32,
                        kind="ExternalInput").ap()
    ht = nc.dram_tensor("ht", [FEATURE_DIM, NUM_HASHES], mybir.dt.float32,
                        kind="ExternalInput").ap()
    n_groups = cols // W
    acc_out = nc.dram_tensor("acc", [NUM_HASHES, n_groups], mybir.dt.float32,
                             kind="ExternalOutput").ap()

    n_blocks = cols // TB
    banks_per_block = TB // 512
    groups_per_bank = 512 // W

    with tile.TileContext(nc) as tc:
        with (
            tc.tile_pool(name="singles", bufs=1) as singles,
            tc.tile_pool(name="xtiles", bufs=3) as xtiles,
            tc.tile_pool(name="psum", bufs=8, space="PSUM") as psum,
        ):
            ht_sb = singles.tile([128, 2, NUM_HASHES], mybir.dt.float32)
            nc.sync.dma_start(out=ht_sb[:, 0, :], in_=ht[0:128, :])
            nc.sync.dma_start(out=ht_sb[:, 1, :], in_=ht[128:256, :])

            acc_sb = singles.tile([128, n_groups], mybir.dt.float32)

            for b in range(n_blocks):
                x_tile = xtiles.tile([128, 2, TB], mybir.dt.float32)
                sl = slice(b * TB, (b + 1) * TB)
                nc.sync.dma_start(out=x_tile[:, 0, :], in_=xt[0:128, sl])
                nc.sync.dma_start(out=x_tile[:, 1, :], in_=xt[128:256, sl])

                for k in range(banks_per_block):
                    hv = psum.tile([128, 512], mybir.dt.float32)
                    ksl = slice(k * 512, (k + 1) * 512)
                    nc.tensor.matmul(hv, ht_sb[:, 0, :], x_tile[:, 0, ksl],
                                     start=True, stop=False)
                    nc.tensor.matmul(hv, ht_sb[:, 1, :], x_tile[:, 1, ksl],
                                     start=False, stop=True)
                    g0 = b * banks_per_block * groups_per_bank + k * groups_per_bank
                    nc.vector.tensor_reduce(
                        out=acc_sb[:, g0:g0 + groups_per_bank],
                        in_=hv.rearrange("p (g w) -> p g w", w=W),
                        axis=mybir.AxisListType.X,
                        op=mybir.AluOpType.min,
                    )

            nc.sync.dma_start(out=acc_out, in_=acc_sb)

    nc.compile()
    return nc


def kernel(x, batch, num_segments, hash_matrices):
    from concourse import bass_utils

    x = np.ascontiguousarray(np.asarray(x), dtype=np.float32)
    batch = np.asarray(batch).astype(np.int64).ravel()
    num_segments = int(num_segments)
    hm = np.asarray(hash_matrices, dtype=np.float32)

    n_nodes = x.shape[0]
    assert x.shape[1] == FEATURE_DIM and hm.shape == (NUM_HASHES, FEATURE_DIM)

    # --- host: group construction -----------------------------------------
    counts = np.bincount(batch, minlength=num_segments)
    order = np.argsort(batch, kind="stable")  # contiguous runs per segment
    seg_starts = np.zeros(num_segments + 1, dtype=np.int64)
    np.cumsum(counts, out=seg_starts[1:])

    grp_seg = []    # segment id per group
    grp_start = []  # start offset in `order`
    grp_len = []    # real nodes in the group (rest padded)
    for s in range(num_segments):
        n_s = int(counts[s])
        if n_s == 0:
            continue
        base = int(seg_starts[s])
        for o in range(0, n_s, W):
            grp_seg.append(s)
            grp_start.append(base + o)
            grp_len.append(min(W, n_s - o))
    g_tot = len(grp_seg)

    # groups per core: uniform, and core cols divisible by TB
    gpc = -(-g_tot // N_CORES)
    per_tb = TB // W
    gpc = -(-gpc // per_tb) * per_tb
    cols = gpc * W

    grp_seg = np.asarray(grp_seg + [-1] * (gpc * N_CORES - g_tot), dtype=np.int64)
    grp_start = np.asarray(grp_start + [0] * (gpc * N_CORES - g_tot), dtype=np.int64)
    grp_len = np.asarray(grp_len + [W] * (gpc * N_CORES - g_tot), dtype=np.int64)

    # per-group node indices, padding with repeats from the same group
    offs = np.arange(W, dtype=np.int64)[None, :] % grp_len[:, None]
    idx = order[grp_start[:, None] + offs]          # [n_groups_tot, W]
    idx = idx.reshape(N_CORES, cols)

    # --- host: build per-core transposed shards ---------------------------
    ht_in = np.ascontiguousarray(hm.T)              # [256, 128]
    in_maps = []
    for c in range(N_CORES):
        xt_c = np.ascontiguousarray(x[idx[c]].T)    # [256, cols]
        in_maps.append({"xt": xt_c, "ht": ht_in})

    # --- device ------------------------------------------------------------
    if cols not in _compiled_cache:
        _compiled_cache[cols] = _build_program(cols)
    nc = _compiled_cache[cols]

    res = bass_utils.run_bass_kernel_spmd(
        nc, in_maps, core_ids=list(range(N_CORES)), trace=False
    )

    # --- host: combine -----------------------------------------------------
    sketch = np.full((num_segments, NUM_HASHES), np.inf, dtype=np.float32)
    grp_seg = grp_seg.reshape(N_CORES, gpc)
    for c in range(N_CORES):
        acc = res.results[c]["acc"]                 # [128, gpc]
        valid = grp_seg[c] >= 0
        np.minimum.at(sketch, grp_seg[c][valid], acc.T[valid])
    sketch[counts == 0] = 0.0
    return sketch


# revision 3
# speedup vs baseline: 1.2178x; 1.2178x over previous
"""MinHash sketch kernel for Trainium2 (8 NeuronCores, Bass/Tile).

Computes: sketch = segment_min(x @ hash_matrices.T, batch) over 512 segments,
with empty segments set to 0.  x: [N, 256] f32, batch: [N] sorted int64,
hash_matrices: [128, 256] f32 -> out [512, 128] f32.

Strategy (data-parallel over nodes):
  * Host groups each segment's nodes into 128-wide "groups" (the last group of
    a segment is padded with duplicate nodes from the same segment, which is
    min-neutral).  Groups are distributed contiguously over the 8 cores and
    padded with dummy groups so every core runs the identical program.
  * Each core's node shard is laid out TRANSPOSED on host ([256, cols]) so the
    contraction dim (features) sits on SBUF partitions - no on-device
    transpose needed.
  * Device: hv[h, n] accumulated in PSUM over the two 128-feature chunks
    (h = 128 hashes on partitions), then one segmented reduce_min per PSUM
    bank ([128, 4, 128] -> [128, 4]) into an SBUF accumulator [128, G]; a
    single DMA writes it out.
  * Host: scatter-min each (core, group) column back to its segment, zero
    empty segments.

Precision/speed scheme for the matmul (SCHEME):
  * "hilo":  x and H split into bf16 hi+lo pairs on host; 3-term product
             (hi*hi + hi*lo + lo*hi) at full PE rate.  ~4e-6 rel error,
             same DMA bytes as fp32.
  * "f32r":  x, H rounded to FP32R (1-8-11) on host; single-term matmul at
             full PE rate.  ~1.5e-4 rel error.
  * "fp32":  exact fp32 matmul; PE runs at 1/4 rate (2 half-speed passes).
"""

import sys

if "/opt/trn_rl_repo" not in sys.path:
    sys.path.insert(0, "/opt/trn_rl_repo")

import numpy as np

SCHEME = "hilo"
N_CORES = 8
W = 128          # nodes per group (reduce_min granularity)
TB = 2048        # columns per DMA block
NUM_HASHES = 128
FEATURE_DIM = 256

_compiled_cache = {}


def round_fp32r(a):
    """Round-to-nearest-even to FP32R (1-8-11); low 12 mantissa bits zero."""
    b = np.ascontiguousarray(a, dtype=np.float32).view(np.uint32)
    low = b & np.uint32(0xFFF)
    b2 = b & np.uint32(0xFFFFF000)
    up = (low > 0x800) | ((low == 0x800) & (((b2 >> 12) & 1) == 1))
    return (b2 + (up.astype(np.uint32) << 12)).view(np.float32)


def _build_program(cols, scheme):
    """Build + compile the single-core Bass program for a shard of `cols`
    node-columns (cols % TB == 0)."""
    import concourse.bacc as bacc
    import concourse.mybir as mybir
    import concourse.tile as tile

    nc = bacc.Bacc("TRN2", target_bir_lowering=False, debug=False,
                   num_devices=N_CORES)

    n_groups = cols // W
    n_blocks = cols // TB
    banks_per_block = TB // 512
    groups_per_bank = 512 // W

    if scheme == "hilo":
        xdt = mybir.dt.bfloat16
        x_names = ["xhi", "xlo"]
        h_names = ["hhi", "hlo"]
    else:
        xdt = mybir.dt.float32r if scheme == "f32r" else mybir.dt.float32
        x_names = ["xt"]
        h_names = ["ht"]

    x_in = {n: nc.dram_tensor(n, [FEATURE_DIM, cols], xdt,
                              kind="ExternalInput").ap() for n in x_names}
    h_in = {n: nc.dram_tensor(n, [FEATURE_DIM, NUM_HASHES], xdt,
                              kind="ExternalInput").ap() for n in h_names}
    acc_out = nc.dram_tensor("acc", [NUM_HASHES, n_groups], mybir.dt.float32,
                             kind="ExternalOutput").ap()

    with tile.TileContext(nc) as tc:
        with (
            tc.tile_pool(name="singles", bufs=1) as singles,
            tc.tile_pool(name="xtiles", bufs=3) as xtiles,
            tc.tile_pool(name="psum", bufs=8, space="PSUM") as psum,
        ):
            h_sb = {}
            for n in h_names:
                t = singles.tile([128, 2, NUM_HASHES], xdt, tag=f"h_{n}")
                nc.sync.dma_start(out=t[:, 0, :], in_=h_in[n][0:128, :])
                nc.sync.dma_start(out=t[:, 1, :], in_=h_in[n][128:256, :])
                h_sb[n] = t

            acc_sb = singles.tile([128, n_groups], mybir.dt.float32)

            # (weight tensor, chunk, rhs tensor) per accumulation term
            if scheme == "hilo":
                phases = [("hhi", 0, "xhi"), ("hhi", 1, "xhi"),
                          ("hlo", 0, "xhi"), ("hlo", 1, "xhi"),
                          ("hhi", 0, "xlo"), ("hhi", 1, "xlo")]
            else:
                phases = [(h_names[0], 0, x_names[0]),
                          (h_names[0], 1, x_names[0])]

            for b in range(n_blocks):
                sl = slice(b * TB, (b + 1) * TB)
                x_sb = {}
                for n in x_names:
                    t = xtiles.tile([128, 2, TB], xdt, tag=f"x_{n}")
                    nc.sync.dma_start(out=t[:, 0, :], in_=x_in[n][0:128, sl])
                    nc.sync.dma_start(out=t[:, 1, :], in_=x_in[n][128:256, sl])
                    x_sb[n] = t

                hv = []
                for _k in range(banks_per_block):
                    hv_bank = psum.tile([128, 512], mybir.dt.float32, tag="hv")
                    hv.append(hv_bank)
                for p, (hn, chunk, xn) in enumerate(phases):
                    for k in range(banks_per_block):
                        ksl = slice(k * 512, (k + 1) * 512)
                        nc.tensor.matmul(hv[k], h_sb[hn][:, chunk, :],
                                         x_sb[xn][:, chunk, ksl],
                                         start=(p == 0),
                                         stop=(p == len(phases) - 1))

                for k in range(banks_per_block):
                    g0 = (b * banks_per_block + k) * groups_per_bank
                    nc.vector.tensor_reduce(
                        out=acc_sb[:, g0:g0 + groups_per_bank],
                        in_=hv[k].rearrange("p (g w) -> p g w", w=W),
                        axis=mybir.AxisListType.X,
                        op=mybir.AluOpType.min,
                    )

            nc.sync.dma_start(out=acc_out, in_=acc_sb)

    nc.compile()
    return nc


def kernel(x, batch, num_segments, hash_matrices):
    import ml_dtypes
    from concourse import bass_utils

    x = np.ascontiguousarray(np.asarray(x), dtype=np.float32)
    batch = np.asarray(batch).astype(np.int64).ravel()
    num_segments = int(num_segments)
    hm = np.asarray(hash_matrices, dtype=np.float32)

    assert x.shape[1] == FEATURE_DIM and hm.shape == (NUM_HASHES, FEATURE_DIM)

    # --- host: group construction -----------------------------------------
    counts = np.bincount(batch, minlength=num_segments)
    order = np.argsort(batch, kind="stable")  # contiguous runs per segment
    seg_starts = np.zeros(num_segments + 1, dtype=np.int64)
    np.cumsum(counts, out=seg_starts[1:])

    grp_seg = []    # segment id per group
    grp_start = []  # start offset in `order`
    grp_len = []    # real nodes in the group (rest padded)
    for s in range(num_segments):
        n_s = int(counts[s])
        if n_s == 0:
            continue
        base = int(seg_starts[s])
        for o in range(0, n_s, W):
            grp_seg.append(s)
            grp_start.append(base + o)
            grp_len.append(min(W, n_s - o))
    g_tot = len(grp_seg)

    # groups per core: uniform, and core cols divisible by TB
    gpc = -(-g_tot // N_CORES)
    per_tb = TB // W
    gpc = -(-gpc // per_tb) * per_tb
    cols = gpc * W

    pad = gpc * N_CORES - g_tot
    grp_seg = np.asarray(grp_seg + [-1] * pad, dtype=np.int64)
    grp_start = np.asarray(grp_start + [0] * pad, dtype=np.int64)
    grp_len = np.asarray(grp_len + [W] * pad, dtype=np.int64)

    # per-group node indices, padding with repeats from the same group
    offs = np.arange(W, dtype=np.int64)[None, :] % grp_len[:, None]
    idx = order[grp_start[:, None] + offs]          # [n_groups_tot, W]
    idx = idx.reshape(N_CORES, cols)

    # --- host: build per-core shards ---------------------------------------
    bf16 = ml_dtypes.bfloat16
    in_maps = []
    if SCHEME == "hilo":
        hhi = hm.T.astype(bf16)
        hlo = (hm.T - hhi.astype(np.float32)).astype(bf16)
        hhi = np.ascontiguousarray(hhi)
        hlo = np.ascontiguousarray(hlo)
        for c in range(N_CORES):
            xt = x[idx[c]].T                         # [256, cols] f32
            xhi = xt.astype(bf16)
            xlo = (xt - xhi.astype(np.float32)).astype(bf16)
            in_maps.append({"xhi": np.ascontiguousarray(xhi),
                            "xlo": np.ascontiguousarray(xlo),
                            "hhi": hhi, "hlo": hlo})
    elif SCHEME == "f32r":
        ht = round_fp32r(np.ascontiguousarray(hm.T))
        for c in range(N_CORES):
            in_maps.append({"xt": round_fp32r(np.ascontiguousarray(x[idx[c]].T)),
                            "ht": ht})
    else:
        ht = np.ascontiguousarray(hm.T)
        for c in range(N_CORES):
            in_maps.append({"xt": np.ascontiguousarray(x[idx[c]].T), "ht": ht})

    # --- device ------------------------------------------------------------
    key = (cols, SCHEME)
    if key not in _compiled_cache:
        _compiled_cache[key] = _build_program(cols, SCHEME)
    nc = _compiled_cache[key]

    res = bass_utils.run_bass_kernel_spmd(
        nc, in_maps, core_ids=list(range(N_CORES)), trace=False
    )

    # --- host: combine -----------------------------------------------------
    sketch = np.full((num_segments, NUM_HASHES), np.inf, dtype=np.float32)
    grp_seg = grp_seg.reshape(N_CORES, gpc)
    for c in range(N_CORES):
        acc = res.results[c]["acc"]                 # [128, gpc]
        valid = grp_seg[c] >= 0
        np.minimum.at(sketch, grp_seg[c][valid], acc.T[valid])
    sketch[counts == 0] = 0.0
    return sketch


# revision 18
# speedup vs baseline: 1.2839x; 1.0543x over previous
"""MinHash sketch kernel for Trainium2 (8 NeuronCores, Bass/Tile).

Computes: sketch = segment_min(x @ hash_matrices.T, batch) over 512 segments,
with empty segments set to 0.  x: [N, 256] f32, batch: [N] sorted int64,
hash_matrices: [128, 256] f32 -> out [512, 128] f32.

Strategy (data-parallel over nodes):
  * Host groups each segment's nodes into 128-wide "groups" (the last group of
    a segment is padded with duplicate nodes from the same segment, which is
    min-neutral).  Groups are distributed contiguously over the 8 cores and
    padded with dummy groups so every core runs the identical program.
  * Each core's node shard is laid out TRANSPOSED on host ([256, cols]) so the
    contraction dim (features) sits on SBUF partitions - no on-device
    transpose needed.
  * Device: hv[h, n] accumulated in PSUM over the two 128-feature chunks
    (h = 128 hashes on partitions), then one segmented reduce_min per PSUM
    bank ([128, 4, 128] -> [128, 4]) into an SBUF accumulator [128, G]; a
    single DMA writes it out.
  * Host: scatter-min each (core, group) column back to its segment, zero
    empty segments.

Precision/speed scheme for the matmul (SCHEME):
  * "hilo":  x and H split into bf16 hi+lo pairs on host; 3-term product
             (hi*hi + hi*lo + lo*hi) at full PE rate.  ~4e-6 rel error,
             same DMA bytes as fp32.
  * "f32r":  x, H rounded to FP32R (1-8-11) on host; single-term matmul at
             full PE rate.  ~1.5e-4 rel error.
  * "fp32":  exact fp32 matmul; PE runs at 1/4 rate (2 half-speed passes).
"""

import sys

if "/opt/trn_rl_repo" not in sys.path:
    sys.path.insert(0, "/opt/trn_rl_repo")

import numpy as np

SCHEME = "hilo"
N_CORES = 8
W = 32           # nodes per group (reduce_min granularity)
BANK = 512       # PSUM bank width (fp32)
TB = 2048        # columns per full DMA block
NUM_HASHES = 128
FEATURE_DIM = 256

_compiled_cache = {}


def round_fp32r(a):
    """Round-to-nearest-even to FP32R (1-8-11); low 12 mantissa bits zero."""
    b = np.ascontiguousarray(a, dtype=np.float32).view(np.uint32)
    low = b & np.uint32(0xFFF)
    b2 = b & np.uint32(0xFFFFF000)
    up = (low > 0x800) | ((low == 0x800) & (((b2 >> 12) & 1) == 1))
    return (b2 + (up.astype(np.uint32) << 12)).view(np.float32)


def _build_program(cols, scheme):
    """Build + compile the single-core Bass program for a shard of `cols`
    node-columns (cols % TB == 0)."""
    import concourse.bacc as bacc
    import concourse.mybir as mybir
    import concourse.tile as tile

    nc = bacc.Bacc("TRN2", target_bir_lowering=False, debug=False,
                   num_devices=N_CORES)

    assert cols % BANK == 0
    n_groups = cols // W
    groups_per_bank = BANK // W
    # full TB-wide blocks, then a descending tail (1024, 512, 512) so the
    # post-final-DMA compute tail is short
    rest = cols
    block_widths = []
    while rest > 2 * TB:
        block_widths.append(TB)
        rest -= TB
    for piece in (TB, 1024, 1024, 512, 512, 512, 512):
        if rest >= piece and rest - piece != BANK // 2:
            block_widths.append(piece)
            rest -= piece
        if rest == 0:
            break
    assert rest == 0 and sum(block_widths) == cols, (cols, rest)

    if scheme == "hilo":
        xdt = mybir.dt.bfloat16
        x_names = ["xhi", "xlo"]
        h_names = ["hhi", "hlo"]
    else:
        xdt = mybir.dt.float32r if scheme == "f32r" else mybir.dt.float32
        x_names = ["xt"]
        h_names = ["ht"]

    x_in = {n: nc.dram_tensor(n, [FEATURE_DIM, cols], xdt,
                              kind="ExternalInput").ap() for n in x_names}
    h_in = {n: nc.dram_tensor(n, [FEATURE_DIM, NUM_HASHES], xdt,
                              kind="ExternalInput").ap() for n in h_names}
    acc_out = nc.dram_tensor("acc", [NUM_HASHES, n_groups], mybir.dt.float32,
                             kind="ExternalOutput").ap()

    with tile.TileContext(nc) as tc:
        with (
            tc.tile_pool(name="singles", bufs=1) as singles,
            tc.tile_pool(name="xtiles", bufs=3) as xtiles,
            tc.tile_pool(name="psum", bufs=8, space="PSUM") as psum,
        ):
            acc_sb = singles.tile([128, n_groups], mybir.dt.float32)
            h_sb = {}
            for n in h_names:
                t = singles.tile([128, 2, NUM_HASHES], xdt, tag=f"h_{n}")
                nc.sync.dma_start(out=t[:, 0, :], in_=h_in[n][0:128, :])
                nc.sync.dma_start(out=t[:, 1, :], in_=h_in[n][128:256, :])
                h_sb[n] = t


            # (weight tensor, chunk, rhs tensor) per accumulation term
            if scheme == "hilo":
                phases = [("hhi", 0, "xhi"), ("hhi", 1, "xhi"),
                          ("hlo", 0, "xhi"), ("hlo", 1, "xhi"),
                          ("hhi", 0, "xlo"), ("hhi", 1, "xlo")]
            else:
                phases = [(h_names[0], 0, x_names[0]),
                          (h_names[0], 1, x_names[0])]

            col0 = 0
            flushed = 0
            flush_step = max(TB, cols // 4)
            flush_at = flush_step
            for tb in block_widths:
                sl = slice(col0, col0 + tb)
                banks_per_block = tb // BANK
                x_sb = {}
                for n in x_names:
                    t = xtiles.tile([128, 2, TB], xdt, tag=f"x_{n}")
                    nc.sync.dma_start(out=t[:, 0, :tb], in_=x_in[n][0:128, sl])
                    nc.sync.dma_start(out=t[:, 1, :tb], in_=x_in[n][128:256, sl])
                    x_sb[n] = t

                hv = []
                for _k in range(banks_per_block):
                    hv_bank = psum.tile([128, BANK], mybir.dt.float32, tag="hv")
                    hv.append(hv_bank)
                for p, (hn, chunk, xn) in enumerate(phases):
                    for k in range(banks_per_block):
                        ksl = slice(k * BANK, (k + 1) * BANK)
                        nc.tensor.matmul(hv[k], h_sb[hn][:, chunk, :],
                                         x_sb[xn][:, chunk, ksl],
                                         start=(p == 0),
                                         stop=(p == len(phases) - 1))

                for k in range(banks_per_block):
                    g0 = (col0 + k * BANK) // W
                    nc.vector.tensor_reduce(
                        out=acc_sb[:, g0:g0 + groups_per_bank],
                        in_=hv[k].rearrange("p (g w) -> p g w", w=W),
                        axis=mybir.AxisListType.X,
                        op=mybir.AluOpType.min,
                    )
                col0 += tb
                # flush finished accumulator ranges on the SWDGE queue (doesn't
                # block the HWDGE input stream); keep only the last block's
                # groups for the final flush so the serial tail is tiny
                if col0 >= flush_at and col0 < cols:
                    g1 = col0 // W
                    nc.gpsimd.dma_start(out=acc_out[:, flushed:g1],
                                        in_=acc_sb[:, flushed:g1])
                    flushed = g1
                    flush_at = col0 + flush_step

            nc.gpsimd.dma_start(out=acc_out[:, flushed:], in_=acc_sb[:, flushed:])

    nc.compile()
    return nc


def kernel(x, batch, num_segments, hash_matrices):
    import ml_dtypes
    from concourse import bass_utils

    x = np.ascontiguousarray(np.asarray(x), dtype=np.float32)
    batch = np.asarray(batch).astype(np.int64).ravel()
    num_segments = int(num_segments)
    hm = np.asarray(hash_matrices, dtype=np.float32)

    assert x.shape[1] == FEATURE_DIM and hm.shape == (NUM_HASHES, FEATURE_DIM)

    # --- host: group construction -----------------------------------------
    counts = np.bincount(batch, minlength=num_segments)
    order = np.argsort(batch, kind="stable")  # contiguous runs per segment
    seg_starts = np.zeros(num_segments + 1, dtype=np.int64)
    np.cumsum(counts, out=seg_starts[1:])

    grp_seg = []    # segment id per group
    grp_start = []  # start offset in `order`
    grp_len = []    # real nodes in the group (rest padded)
    for s in range(num_segments):
        n_s = int(counts[s])
        if n_s == 0:
            continue
        base = int(seg_starts[s])
        for o in range(0, n_s, W):
            grp_seg.append(s)
            grp_start.append(base + o)
            grp_len.append(min(W, n_s - o))
    g_tot = len(grp_seg)

    # groups per core: uniform, and core cols divisible by BANK
    gpc = -(-g_tot // N_CORES)
    per_bank = BANK // W
    gpc = -(-gpc // per_bank) * per_bank
    cols = gpc * W

    pad = gpc * N_CORES - g_tot
    grp_seg = np.asarray(grp_seg + [-1] * pad, dtype=np.int64)
    grp_start = np.asarray(grp_start + [0] * pad, dtype=np.int64)
    grp_len = np.asarray(grp_len + [W] * pad, dtype=np.int64)

    # per-group node indices, padding with repeats from the same group
    offs = np.arange(W, dtype=np.int64)[None, :] % grp_len[:, None]
    idx = order[grp_start[:, None] + offs]          # [n_groups_tot, W]
    idx = idx.reshape(N_CORES, cols)

    # --- host: build per-core shards ---------------------------------------
    bf16 = ml_dtypes.bfloat16
    in_maps = []
    if SCHEME == "hilo":
        hhi = hm.T.astype(bf16)
        hlo = (hm.T - hhi.astype(np.float32)).astype(bf16)
        hhi = np.ascontiguousarray(hhi)
        hlo = np.ascontiguousarray(hlo)
        for c in range(N_CORES):
            xt = x[idx[c]].T                         # [256, cols] f32
            xhi = xt.astype(bf16)
            xlo = (xt - xhi.astype(np.float32)).astype(bf16)
            in_maps.append({"xhi": np.ascontiguousarray(xhi),
                            "xlo": np.ascontiguousarray(xlo),
                            "hhi": hhi, "hlo": hlo})
    elif SCHEME == "f32r":
        ht = round_fp32r(np.ascontiguousarray(hm.T))
        for c in range(N_CORES):
            in_maps.append({"xt": round_fp32r(np.ascontiguousarray(x[idx[c]].T)),
                            "ht": ht})
    else:
        ht = np.ascontiguousarray(hm.T)
        for c in range(N_CORES):
            in_maps.append({"xt": np.ascontiguousarray(x[idx[c]].T), "ht": ht})

    # --- device ------------------------------------------------------------
    key = (cols, SCHEME)
    if key not in _compiled_cache:
        _compiled_cache[key] = _build_program(cols, SCHEME)
    nc = _compiled_cache[key]

    res = bass_utils.run_bass_kernel_spmd(
        nc, in_maps, core_ids=list(range(N_CORES)), trace=False
    )

    # --- host: combine -----------------------------------------------------
    sketch = np.full((num_segments, NUM_HASHES), np.inf, dtype=np.float32)
    grp_seg = grp_seg.reshape(N_CORES, gpc)
    for c in range(N_CORES):
        acc = res.results[c]["acc"]                 # [128, gpc]
        valid = grp_seg[c] >= 0
        np.minimum.at(sketch, grp_seg[c][valid], acc.T[valid])
    sketch[counts == 0] = 0.0
    return sketch


# revision 20
# speedup vs baseline: 1.2871x; 1.0025x over previous
"""MinHash sketch kernel for Trainium2 (8 NeuronCores, Bass/Tile).

Computes: sketch = segment_min(x @ hash_matrices.T, batch) over 512 segments,
with empty segments set to 0.  x: [N, 256] f32, batch: [N] sorted int64,
hash_matrices: [128, 256] f32 -> out [512, 128] f32.

Strategy (data-parallel over nodes):
  * Host groups each segment's nodes into W=32-wide "groups" (the last group
    of a segment is padded with duplicate nodes from the same segment, which
    is min-neutral).  Groups are distributed contiguously over the 8 cores
    and padded with dummy groups so every core runs the identical program.
  * Each core's node shard is laid out TRANSPOSED on host ([256, cols]) so the
    contraction dim (features) sits on SBUF partitions - no on-device
    transpose needed (fp32 DMA transpose doesn't exist and PE transpose would
    make the tensor engine the bottleneck).
  * Device: stream x in 2048-column blocks (descending-width tail so the
    post-final-DMA compute tail is short); hv[h, n] accumulated in PSUM over
    the two 128-feature chunks (h = 128 hashes on partitions), then one
    segmented reduce_min per PSUM bank ([128, 16, 32] -> [128, 16]) into an
    SBUF accumulator [128, G], flushed progressively to DRAM on the SWDGE
    queue so it never blocks the HWDGE input stream.
  * Host: scatter-min each (core, group) column back to its segment, zero
    empty segments.
  * No collective needed: group->segment mapping is host-side, so per-core
    partial sketches are min-combined on the host during unsharding.
  * Cost model (TimelineSim): ~194 us/core, DMA-bound at ~358 GB/s HBM
    (65 MB/core input); PE ~85% busy, DVE ~42%.

Precision/speed scheme for the matmul (SCHEME):
  * "hilo":  x and H split into bf16 hi+lo pairs on host; 3-term product
             (hi*hi + hi*lo + lo*hi) at full PE rate.  ~4e-6 rel error,
             same DMA bytes as fp32.
  * "f32r":  x, H rounded to FP32R (1-8-11) on host; single-term matmul at
             full PE rate.  ~1.5e-4 rel error.
  * "fp32":  exact fp32 matmul; PE runs at 1/4 rate (2 half-speed passes).
"""

import sys

if "/opt/trn_rl_repo" not in sys.path:
    sys.path.insert(0, "/opt/trn_rl_repo")

import numpy as np

SCHEME = "hilo"
N_CORES = 8
W = 32           # nodes per group (reduce_min granularity)
BANK = 512       # PSUM bank width (fp32)
TB = 2048        # columns per full DMA block
NUM_HASHES = 128
FEATURE_DIM = 256

_compiled_cache = {}


def round_fp32r(a):
    """Round-to-nearest-even to FP32R (1-8-11); low 12 mantissa bits zero."""
    b = np.ascontiguousarray(a, dtype=np.float32).view(np.uint32)
    low = b & np.uint32(0xFFF)
    b2 = b & np.uint32(0xFFFFF000)
    up = (low > 0x800) | ((low == 0x800) & (((b2 >> 12) & 1) == 1))
    return (b2 + (up.astype(np.uint32) << 12)).view(np.float32)


def _build_program(cols, scheme):
    """Build + compile the single-core Bass program for a shard of `cols`
    node-columns (cols % TB == 0)."""
    import concourse.bacc as bacc
    import concourse.mybir as mybir
    import concourse.tile as tile

    nc = bacc.Bacc("TRN2", target_bir_lowering=False, debug=False,
                   num_devices=N_CORES)

    assert cols % BANK == 0
    n_groups = cols // W
    groups_per_bank = BANK // W
    # full TB-wide blocks, then a descending tail (1024, 512, 512) so the
    # post-final-DMA compute tail is short
    rest = cols
    block_widths = []
    while rest > 2 * TB:
        block_widths.append(TB)
        rest -= TB
    for piece in (TB, 1024, 512, 512, 512, 512, 512, 512):
        if rest >= piece and rest - piece != BANK // 2:
            block_widths.append(piece)
            rest -= piece
        if rest == 0:
            break
    assert rest == 0 and sum(block_widths) == cols, (cols, rest)

    if scheme == "hilo":
        xdt = mybir.dt.bfloat16
        x_names = ["xhi", "xlo"]
        h_names = ["hhi", "hlo"]
    else:
        xdt = mybir.dt.float32r if scheme == "f32r" else mybir.dt.float32
        x_names = ["xt"]
        h_names = ["ht"]

    x_in = {n: nc.dram_tensor(n, [FEATURE_DIM, cols], xdt,
                              kind="ExternalInput").ap() for n in x_names}
    h_in = {n: nc.dram_tensor(n, [FEATURE_DIM, NUM_HASHES], xdt,
                              kind="ExternalInput").ap() for n in h_names}
    acc_out = nc.dram_tensor("acc", [NUM_HASHES, n_groups], mybir.dt.float32,
                             kind="ExternalOutput").ap()

    with tile.TileContext(nc) as tc:
        with (
            tc.tile_pool(name="singles", bufs=1) as singles,
            tc.tile_pool(name="xtiles", bufs=3) as xtiles,
            tc.tile_pool(name="psum", bufs=8, space="PSUM") as psum,
        ):
            acc_sb = singles.tile([128, n_groups], mybir.dt.float32)
            h_sb = {}
            for n in h_names:
                t = singles.tile([128, 2, NUM_HASHES], xdt, tag=f"h_{n}")
                nc.sync.dma_start(out=t[:, 0, :], in_=h_in[n][0:128, :])
                nc.sync.dma_start(out=t[:, 1, :], in_=h_in[n][128:256, :])
                h_sb[n] = t


            # (weight tensor, chunk, rhs tensor) per accumulation term
            if scheme == "hilo":
                phases = [("hhi", 0, "xhi"), ("hhi", 1, "xhi"),
                          ("hlo", 0, "xhi"), ("hlo", 1, "xhi"),
                          ("hhi", 0, "xlo"), ("hhi", 1, "xlo")]
            else:
                phases = [(h_names[0], 0, x_names[0]),
                          (h_names[0], 1, x_names[0])]

            col0 = 0
            flushed = 0
            flush_step = max(TB, cols // 4)
            flush_at = flush_step
            for tb in block_widths:
                sl = slice(col0, col0 + tb)
                banks_per_block = tb // BANK
                x_sb = {}
                for n in x_names:
                    t = xtiles.tile([128, 2, TB], xdt, tag=f"x_{n}")
                    nc.sync.dma_start(out=t[:, 0, :tb], in_=x_in[n][0:128, sl])
                    nc.sync.dma_start(out=t[:, 1, :tb], in_=x_in[n][128:256, sl])
                    x_sb[n] = t

                hv = []
                for _k in range(banks_per_block):
                    hv_bank = psum.tile([128, BANK], mybir.dt.float32, tag="hv")
                    hv.append(hv_bank)
                for p, (hn, chunk, xn) in enumerate(phases):
                    for k in range(banks_per_block):
                        ksl = slice(k * BANK, (k + 1) * BANK)
                        nc.tensor.matmul(hv[k], h_sb[hn][:, chunk, :],
                                         x_sb[xn][:, chunk, ksl],
                                         start=(p == 0),
                                         stop=(p == len(phases) - 1))

                for k in range(banks_per_block):
                    g0 = (col0 + k * BANK) // W
                    nc.vector.tensor_reduce(
                        out=acc_sb[:, g0:g0 + groups_per_bank],
                        in_=hv[k].rearrange("p (g w) -> p g w", w=W),
                        axis=mybir.AxisListType.X,
                        op=mybir.AluOpType.min,
                    )
                col0 += tb
                # flush finished accumulator ranges on the SWDGE queue (doesn't
                # block the HWDGE input stream); keep only the last block's
                # groups for the final flush so the serial tail is tiny
                if col0 >= flush_at and col0 < cols:
                    g1 = col0 // W
                    nc.gpsimd.dma_start(out=acc_out[:, flushed:g1],
                                        in_=acc_sb[:, flushed:g1])
                    flushed = g1
                    flush_at = col0 + flush_step

            nc.gpsimd.dma_start(out=acc_out[:, flushed:], in_=acc_sb[:, flushed:])

    nc.compile()
    return nc


def kernel(x, batch, num_segments, hash_matrices):
    import ml_dtypes
    from concourse import bass_utils

    x = np.ascontiguousarray(np.asarray(x), dtype=np.float32)
    batch = np.asarray(batch).astype(np.int64).ravel()
    num_segments = int(num_segments)
    hm = np.asarray(hash_matrices, dtype=np.float32)

    assert x.shape[1] == FEATURE_DIM and hm.shape == (NUM_HASHES, FEATURE_DIM)

    # --- host: group construction -----------------------------------------
    counts = np.bincount(batch, minlength=num_segments)
    order = np.argsort(batch, kind="stable")  # contiguous runs per segment
    seg_starts = np.zeros(num_segments + 1, dtype=np.int64)
    np.cumsum(counts, out=seg_starts[1:])

    grp_seg = []    # segment id per group
    grp_start = []  # start offset in `order`
    grp_len = []    # real nodes in the group (rest padded)
    for s in range(num_segments):
        n_s = int(counts[s])
        if n_s == 0:
            continue
        base = int(seg_starts[s])
        for o in range(0, n_s, W):
            grp_seg.append(s)
            grp_start.append(base + o)
            grp_len.append(min(W, n_s - o))
    g_tot = len(grp_seg)

    # groups per core: uniform, and core cols divisible by BANK
    gpc = -(-g_tot // N_CORES)
    per_bank = BANK // W
    gpc = -(-gpc // per_bank) * per_bank
    cols = gpc * W

    pad = gpc * N_CORES - g_tot
    grp_seg = np.asarray(grp_seg + [-1] * pad, dtype=np.int64)
    grp_start = np.asarray(grp_start + [0] * pad, dtype=np.int64)
    grp_len = np.asarray(grp_len + [W] * pad, dtype=np.int64)

    # per-group node indices, padding with repeats from the same group
    offs = np.arange(W, dtype=np.int64)[None, :] % grp_len[:, None]
    idx = order[grp_start[:, None] + offs]          # [n_groups_tot, W]
    idx = idx.reshape(N_CORES, cols)

    # --- host: build per-core shards ---------------------------------------
    bf16 = ml_dtypes.bfloat16
    in_maps = []
    if SCHEME == "hilo":
        hhi = hm.T.astype(bf16)
        hlo = (hm.T - hhi.astype(np.float32)).astype(bf16)
        hhi = np.ascontiguousarray(hhi)
        hlo = np.ascontiguousarray(hlo)
        for c in range(N_CORES):
            xt = x[idx[c]].T                         # [256, cols] f32
            xhi = xt.astype(bf16)
            xlo = (xt - xhi.astype(np.float32)).astype(bf16)
            in_maps.append({"xhi": np.ascontiguousarray(xhi),
                            "xlo": np.ascontiguousarray(xlo),
                            "hhi": hhi, "hlo": hlo})
    elif SCHEME == "f32r":
        ht = round_fp32r(np.ascontiguousarray(hm.T))
        for c in range(N_CORES):
            in_maps.append({"xt": round_fp32r(np.ascontiguousarray(x[idx[c]].T)),
                            "ht": ht})
    else:
        ht = np.ascontiguousarray(hm.T)
        for c in range(N_CORES):
            in_maps.append({"xt": np.ascontiguousarray(x[idx[c]].T), "ht": ht})

    # --- device ------------------------------------------------------------
    key = (cols, SCHEME)
    if key not in _compiled_cache:
        _compiled_cache[key] = _build_program(cols, SCHEME)
    nc = _compiled_cache[key]

    res = bass_utils.run_bass_kernel_spmd(
        nc, in_maps, core_ids=list(range(N_CORES)), trace=False
    )

    # --- host: combine -----------------------------------------------------
    sketch = np.full((num_segments, NUM_HASHES), np.inf, dtype=np.float32)
    grp_seg = grp_seg.reshape(N_CORES, gpc)
    for c in range(N_CORES):
        acc = res.results[c]["acc"]                 # [128, gpc]
        valid = grp_seg[c] >= 0
        np.minimum.at(sketch, grp_seg[c][valid], acc.T[valid])
    sketch[counts == 0] = 0.0
    return sketch


# revision 22
# speedup vs baseline: 1.2983x; 1.0087x over previous
"""MinHash sketch kernel for Trainium2 (8 NeuronCores, Bass/Tile).

Computes: sketch = segment_min(x @ hash_matrices.T, batch) over 512 segments,
with empty segments set to 0.  x: [N, 256] f32, batch: [N] sorted int64,
hash_matrices: [128, 256] f32 -> out [512, 128] f32.

Strategy (data-parallel over nodes):
  * Host groups each segment's nodes into W=32-wide "groups" (the last group
    of a segment is padded with duplicate nodes from the same segment, which
    is min-neutral).  Groups are distributed contiguously over the 8 cores
    and padded with dummy groups so every core runs the identical program.
  * Each core's node shard is laid out TRANSPOSED on host ([256, cols]) so the
    contraction dim (features) sits on SBUF partitions - no on-device
    transpose needed (fp32 DMA transpose doesn't exist and PE transpose would
    make the tensor engine the bottleneck).
  * Device: stream x in 2048-column blocks (descending-width tail so the
    post-final-DMA compute tail is short); hv[h, n] accumulated in PSUM over
    the two 128-feature chunks (h = 128 hashes on partitions), then one
    segmented reduce_min per PSUM bank ([128, 16, 32] -> [128, 16]) into an
    SBUF accumulator [128, G], flushed progressively to DRAM on the SWDGE
    queue so it never blocks the HWDGE input stream.
  * Host: scatter-min each (core, group) column back to its segment, zero
    empty segments.
  * No collective needed: group->segment mapping is host-side, so per-core
    partial sketches are min-combined on the host during unsharding.
  * Cost model (TimelineSim): ~192 us/core, DMA-bound at ~358 GB/s HBM
    (65 MB/core input, 1.6% group padding); PE ~85% busy, DVE ~42%.

Precision/speed scheme for the matmul (SCHEME):
  * "hilo":  x and H split into bf16 hi+lo pairs on host; 3-term product
             (hi*hi + hi*lo + lo*hi) at full PE rate.  ~4e-6 rel error,
             same DMA bytes as fp32.
  * "f32r":  x, H rounded to FP32R (1-8-11) on host; single-term matmul at
             full PE rate.  ~1.5e-4 rel error.
  * "fp32":  exact fp32 matmul; PE runs at 1/4 rate (2 half-speed passes).
"""

import sys

if "/opt/trn_rl_repo" not in sys.path:
    sys.path.insert(0, "/opt/trn_rl_repo")

import numpy as np

SCHEME = "hilo"
N_CORES = 8
W = 32           # nodes per group (reduce_min granularity)
BANK = 512       # PSUM bank width (fp32)
TB = 2048        # columns per full DMA block
NUM_HASHES = 128
FEATURE_DIM = 256

_compiled_cache = {}


def round_fp32r(a):
    """Round-to-nearest-even to FP32R (1-8-11); low 12 mantissa bits zero."""
    b = np.ascontiguousarray(a, dtype=np.float32).view(np.uint32)
    low = b & np.uint32(0xFFF)
    b2 = b & np.uint32(0xFFFFF000)
    up = (low > 0x800) | ((low == 0x800) & (((b2 >> 12) & 1) == 1))
    return (b2 + (up.astype(np.uint32) << 12)).view(np.float32)


def _build_program(cols, scheme):
    """Build + compile the single-core Bass program for a shard of `cols`
    node-columns (cols % TB == 0)."""
    import concourse.bacc as bacc
    import concourse.mybir as mybir
    import concourse.tile as tile

    nc = bacc.Bacc("TRN2", target_bir_lowering=False, debug=False,
                   num_devices=N_CORES)

    assert cols % BANK == 0
    n_groups = cols // W
    groups_per_bank = BANK // W
    # full TB-wide blocks, then a descending tail (1024, 512, 512) so the
    # post-final-DMA compute tail is short
    rest = cols
    block_widths = []
    while rest > 2 * TB:
        block_widths.append(TB)
        rest -= TB
    for piece in (TB, 1024, 512, 512, 512, 512, 512, 512):
        if rest >= piece and rest - piece != BANK // 2:
            block_widths.append(piece)
            rest -= piece
        if rest == 0:
            break
    assert rest == 0 and sum(block_widths) == cols, (cols, rest)

    if scheme == "hilo":
        xdt = mybir.dt.bfloat16
        x_names = ["xhi", "xlo"]
        h_names = ["hhi", "hlo"]
    else:
        xdt = mybir.dt.float32r if scheme == "f32r" else mybir.dt.float32
        x_names = ["xt"]
        h_names = ["ht"]

    x_in = {n: nc.dram_tensor(n, [FEATURE_DIM, cols], xdt,
                              kind="ExternalInput").ap() for n in x_names}
    h_in = {n: nc.dram_tensor(n, [FEATURE_DIM, NUM_HASHES], xdt,
                              kind="ExternalInput").ap() for n in h_names}
    acc_out = nc.dram_tensor("acc", [NUM_HASHES, n_groups], mybir.dt.float32,
                             kind="ExternalOutput").ap()

    with tile.TileContext(nc) as tc:
        with (
            tc.tile_pool(name="singles", bufs=1) as singles,
            tc.tile_pool(name="xtiles", bufs=3) as xtiles,
            tc.tile_pool(name="psum", bufs=8, space="PSUM") as psum,
        ):
            acc_sb = singles.tile([128, n_groups], mybir.dt.float32)
            h_sb = {}
            for n in h_names:
                t = singles.tile([128, 2, NUM_HASHES], xdt, tag=f"h_{n}")
                # gpsimd queue: don't delay the first x block on the HWDGE queue
                nc.gpsimd.dma_start(out=t[:, 0, :], in_=h_in[n][0:128, :])
                nc.gpsimd.dma_start(out=t[:, 1, :], in_=h_in[n][128:256, :])
                h_sb[n] = t


            # (weight tensor, chunk, rhs tensor) per accumulation term
            if scheme == "hilo":
                phases = [("hhi", 0, "xhi"), ("hhi", 1, "xhi"),
                          ("hlo", 0, "xhi"), ("hlo", 1, "xhi"),
                          ("hhi", 0, "xlo"), ("hhi", 1, "xlo")]
            else:
                phases = [(h_names[0], 0, x_names[0]),
                          (h_names[0], 1, x_names[0])]

            col0 = 0
            flushed = 0
            flush_step = max(TB, cols // 4)
            flush_at = flush_step
            for tb in block_widths:
                sl = slice(col0, col0 + tb)
                banks_per_block = tb // BANK
                x_sb = {}
                for n in x_names:
                    t = xtiles.tile([128, 2, TB], xdt, tag=f"x_{n}")
                    nc.sync.dma_start(out=t[:, 0, :tb], in_=x_in[n][0:128, sl])
                    nc.sync.dma_start(out=t[:, 1, :tb], in_=x_in[n][128:256, sl])
                    x_sb[n] = t

                hv = []
                for _k in range(banks_per_block):
                    hv_bank = psum.tile([128, BANK], mybir.dt.float32, tag="hv")
                    hv.append(hv_bank)
                for p, (hn, chunk, xn) in enumerate(phases):
                    for k in range(banks_per_block):
                        ksl = slice(k * BANK, (k + 1) * BANK)
                        nc.tensor.matmul(hv[k], h_sb[hn][:, chunk, :],
                                         x_sb[xn][:, chunk, ksl],
                                         start=(p == 0),
                                         stop=(p == len(phases) - 1))

                for k in range(banks_per_block):
                    g0 = (col0 + k * BANK) // W
                    nc.vector.tensor_reduce(
                        out=acc_sb[:, g0:g0 + groups_per_bank],
                        in_=hv[k].rearrange("p (g w) -> p g w", w=W),
                        axis=mybir.AxisListType.X,
                        op=mybir.AluOpType.min,
                    )
                col0 += tb
                # flush finished accumulator ranges on the SWDGE queue (doesn't
                # block the HWDGE input stream); keep only the last block's
                # groups for the final flush so the serial tail is tiny
                if col0 >= flush_at and col0 < cols:
                    g1 = col0 // W
                    nc.gpsimd.dma_start(out=acc_out[:, flushed:g1],
                                        in_=acc_sb[:, flushed:g1])
                    flushed = g1
                    flush_at = col0 + flush_step

            nc.gpsimd.dma_start(out=acc_out[:, flushed:], in_=acc_sb[:, flushed:])

    nc.compile()
    return nc


def kernel(x, batch, num_segments, hash_matrices):
    import ml_dtypes
    from concourse import bass_utils

    x = np.ascontiguousarray(np.asarray(x), dtype=np.float32)
    batch = np.asarray(batch).astype(np.int64).ravel()
    num_segments = int(num_segments)
    hm = np.asarray(hash_matrices, dtype=np.float32)

    assert x.shape[1] == FEATURE_DIM and hm.shape == (NUM_HASHES, FEATURE_DIM)

    # --- host: group construction -----------------------------------------
    counts = np.bincount(batch, minlength=num_segments)
    order = np.argsort(batch, kind="stable")  # contiguous runs per segment
    seg_starts = np.zeros(num_segments + 1, dtype=np.int64)
    np.cumsum(counts, out=seg_starts[1:])

    grp_seg = []    # segment id per group
    grp_start = []  # start offset in `order`
    grp_len = []    # real nodes in the group (rest padded)
    for s in range(num_segments):
        n_s = int(counts[s])
        if n_s == 0:
            continue
        base = int(seg_starts[s])
        for o in range(0, n_s, W):
            grp_seg.append(s)
            grp_start.append(base + o)
            grp_len.append(min(W, n_s - o))
    g_tot = len(grp_seg)

    # groups per core: uniform, and core cols divisible by BANK
    gpc = -(-g_tot // N_CORES)
    per_bank = BANK // W
    gpc = -(-gpc // per_bank) * per_bank
    cols = gpc * W

    pad = gpc * N_CORES - g_tot
    grp_seg = np.asarray(grp_seg + [-1] * pad, dtype=np.int64)
    grp_start = np.asarray(grp_start + [0] * pad, dtype=np.int64)
    grp_len = np.asarray(grp_len + [W] * pad, dtype=np.int64)

    # per-group node indices, padding with repeats from the same group
    offs = np.arange(W, dtype=np.int64)[None, :] % grp_len[:, None]
    idx = order[grp_start[:, None] + offs]          # [n_groups_tot, W]
    idx = idx.reshape(N_CORES, cols)

    # --- host: build per-core shards ---------------------------------------
    bf16 = ml_dtypes.bfloat16
    in_maps = []
    if SCHEME == "hilo":
        hhi = hm.T.astype(bf16)
        hlo = (hm.T - hhi.astype(np.float32)).astype(bf16)
        hhi = np.ascontiguousarray(hhi)
        hlo = np.ascontiguousarray(hlo)
        for c in range(N_CORES):
            xt = x[idx[c]].T                         # [256, cols] f32
            xhi = xt.astype(bf16)
            xlo = (xt - xhi.astype(np.float32)).astype(bf16)
            in_maps.append({"xhi": np.ascontiguousarray(xhi),
                            "xlo": np.ascontiguousarray(xlo),
                            "hhi": hhi, "hlo": hlo})
    elif SCHEME == "f32r":
        ht = round_fp32r(np.ascontiguousarray(hm.T))
        for c in range(N_CORES):
            in_maps.append({"xt": round_fp32r(np.ascontiguousarray(x[idx[c]].T)),
                            "ht": ht})
    else:
        ht = np.ascontiguousarray(hm.T)
        for c in range(N_CORES):
            in_maps.append({"xt": np.ascontiguousarray(x[idx[c]].T), "ht": ht})

    # --- device ------------------------------------------------------------
    key = (cols, SCHEME)
    if key not in _compiled_cache:
        _compiled_cache[key] = _build_program(cols, SCHEME)
    nc = _compiled_cache[key]

    res = bass_utils.run_bass_kernel_spmd(
        nc, in_maps, core_ids=list(range(N_CORES)), trace=False
    )

    # --- host: combine -----------------------------------------------------
    sketch = np.full((num_segments, NUM_HASHES), np.inf, dtype=np.float32)
    grp_seg = grp_seg.reshape(N_CORES, gpc)
    for c in range(N_CORES):
        acc = res.results[c]["acc"]                 # [128, gpc]
        valid = grp_seg[c] >= 0
        np.minimum.at(sketch, grp_seg[c][valid], acc.T[valid])
    sketch[counts == 0] = 0.0
    return sketch
